# revision 1
# baseline (speedup 1.0000x reference)
"""CoNystromAttention Trainium2 kernel.

Shard: 8 cores = 4 batches x 2 head-groups (8 heads each). Per core:
one batch b, 8 heads organized as 4 "pairs" (2 heads = 128 partitions).

Math (reference, with Q=K=V=QKV):
  QKV = X[b].T @ Wq[h].T + bq[h]                       [n=4096, d=64]
  Qt  = window-mean(QKV, 64)                           [m=64, d]
  S   = exp(QKV @ Qt.T / 8)     (Beta; Delta = S.T)    [n, m]
  G   = exp(Qt @ Qt.T / 8)                             [m, m]
  GD  = G / rowsum(G);  V6 = newton_schulz(GD, 6)      (pinv)
  out = diag(1/r) S V6 diag(1/c) S.T QKV,  r=rowsum(S), c=colsum(S)

All big matmuls in float32r (tf32-like, full PE rate at N>=256).
"""

import numpy as np

P = 128
N_TOK = 4096
EMBED = 1024
NPAIR = 4            # head-pairs per core (8 heads)
ECH = EMBED // P     # 8 contraction chunks
XCH = 256            # projection chunk (tokens)
NCHP = N_TOK // XCH  # 16 projection chunks
NCH8 = N_TOK // 512  # 8 ST chunks of 512
TCH = N_TOK // P     # 32 token chunks of 128
NS_ITERS = 6

_CACHE = {}


def _build(global_scale=True):
    import concourse.mybir as mybir
    from concourse import bacc, bass_isa
    from concourse.tile import TileContext
    from concourse.masks import make_identity

    f32 = mybir.dt.float32
    f32r = mybir.dt.float32r
    ALU = mybir.AluOpType
    ACTF = mybir.ActivationFunctionType
    AX = mybir.AxisListType

    nc = bacc.Bacc("TRN2", target_bir_lowering=False, debug=False)
    X = nc.dram_tensor("X", [EMBED, N_TOK], f32, kind="ExternalInput")
    WqT = nc.dram_tensor("WqT", [EMBED, 512], f32, kind="ExternalInput")
    bias = nc.dram_tensor("bias", [512], f32, kind="ExternalInput")
    out_d = nc.dram_tensor("out", [N_TOK, 512], f32, kind="ExternalOutput")
    if global_scale:
        cc_in = nc.dram_tensor("cc_in", [1, 1], f32)
        cc_out = nc.dram_tensor("cc_out", [1, 1], f32, addr_space="Shared")

    with TileContext(nc) as tc, (
        tc.tile_pool(name="big", bufs=1)
    ) as big, tc.tile_pool(name="persist", bufs=1) as pers, tc.tile_pool(
        name="nsv", bufs=1
    ) as nsp:
        # ---------------- persistent tiles ----------------
        ident32 = pers.tile([P, P], f32, tag="ident32")
        make_identity(nc, ident32[:])
        identr = pers.tile([P, P], f32r, tag="identr")
        nc.vector.tensor_copy(identr[:], ident32[:])
        i7 = pers.tile([P, 256], f32, tag="i7")
        i15 = pers.tile([P, 256], f32, tag="i15")
        i13 = pers.tile([P, 256], f32, tag="i13")
        for t, v in ((i7, 7.0), (i15, 15.0), (i13, 13.0)):
            nc.vector.memset(t[:], 0.0)
            nc.vector.tensor_scalar_mul(t[:, :P], ident32[:], v)
        bias_t = pers.tile([P, NPAIR], f32, tag="bias")
        nc.sync.dma_start(bias_t[:], bias.rearrange("(f p) -> p f", p=P))
        zsrc = pers.tile([P, P], f32, tag="zsrc")
        nc.vector.memset(zsrc[:], 0.0)
        qsum = [pers.tile([P, 64], f32, tag=f"qsum{p}", name=f"qsum{p}") for p in range(NPAIR)]
        qkvt = big.tile([P, NPAIR, N_TOK], f32r, tag="qkvt")
        st = big.tile([P, NPAIR, N_TOK], f32r, tag="st")

        # ---------------- phase 1: projection ----------------
        with (
            tc.tile_pool(name="wq", bufs=1) as wqp,
            tc.tile_pool(name="x", bufs=2) as xpool,
            tc.tile_pool(name="x2", bufs=3) as xpool2,
            tc.tile_pool(name="pp", bufs=8, space="PSUM") as pp,
        ):
            wqtr = wqp.tile([P, ECH, 512], f32r, tag="wqtr")
            for half in range(2):
                for ch in range(2):
                    stg = xpool.tile([P, ECH // 2, XCH], f32, tag="xt")
                    nc.sync.dma_start(
                        stg[:],
                        WqT.rearrange("(eo p) hd -> p eo hd", p=P)[
                            :, half * 4:(half + 1) * 4, ch * 256:(ch + 1) * 256
                        ],
                    )
                    nc.vector.tensor_copy(
                        wqtr[:, half * 4:(half + 1) * 4, ch * 256:(ch + 1) * 256],
                        stg[:],
                    )

            xre = X.rearrange("(eo p) n -> p eo n", p=P)
            for c in range(NCHP):
                xrs = []
                for half in range(2):
                    xt = xpool.tile([P, ECH // 2, XCH], f32, tag="xt")
                    nc.sync.dma_start(
                        xt[:],
                        xre[:, half * 4:(half + 1) * 4, c * XCH:(c + 1) * XCH],
                    )
                    xr = xpool2.tile([P, ECH // 2, XCH], f32r, tag="xr")
                    nc.scalar.copy(xr[:], xt[:])
                    xrs.append(xr)
                for p in range(NPAIR):
                    ps = pp.tile([P, XCH], f32, tag="proj")
                    for e in range(ECH):
                        nc.tensor.matmul(
                            ps[:],
                            wqtr[:, e, p * P:(p + 1) * P],
                            xrs[e // 4][:, e % 4, :],
                            start=(e == 0),
                            stop=(e == ECH - 1),
                        )
                    nc.vector.tensor_scalar_add(
                        qkvt[:, p, c * XCH:(c + 1) * XCH], ps[:], bias_t[:, p:p + 1]
                    )
                    # landmark partial sums (pre-rounding, no bias): 4 windows/chunk
                    nc.vector.reduce_sum(
                        qsum[p][:, c * 4:(c + 1) * 4],
                        ps[:].rearrange("p (w t) -> p w t", t=64),
                        axis=AX.X,
                    )

        # ---------------- phase 2 ----------------
        with (
            tc.tile_pool(name="wk", bufs=4) as wk,
            tc.tile_pool(name="sn", bufs=4) as snp,
            tc.tile_pool(name="nsps", bufs=3, space="PSUM") as nsps,
            tc.tile_pool(name="trps", bufs=3, space="PSUM") as trps,
            tc.tile_pool(name="mps", bufs=1, space="PSUM") as mps,
        ):
            # landmarks (Qt~ = qsum/64 + bias), block-diagonal per pair
            blkq = []
            for p in range(NPAIR):
                bq_t = pers.tile([P, P], f32r, tag=f"blkq{p}")
                nc.vector.tensor_copy(bq_t[0:64, 64:128], zsrc[0:64, 0:64])
                nc.vector.tensor_copy(bq_t[64:128, 0:64], zsrc[0:64, 0:64])
                nc.vector.tensor_scalar(
                    bq_t[0:64, 0:64], qsum[p][0:64, :], 1.0 / 64,
                    bias_t[0:64, p:p + 1], ALU.mult, ALU.add,
                )
                nc.vector.tensor_scalar(
                    bq_t[64:128, 64:128], qsum[p][64:128, :], 1.0 / 64,
                    bias_t[64:128, p:p + 1], ALU.mult, ALU.add,
                )
                blkq.append(bq_t)

            # Gamma -> GD -> Newton-Schulz init
            if global_scale:
                gstage = pers.tile([1, 2 * NPAIR], f32, tag="gstage")
                ones_row = pers.tile([1, P], f32, tag="ones_row")
                nc.vector.memset(ones_row[:], 1.0)
            vstate = []
            for p in range(NPAIR):
                psg = nsps.tile([P, 256], f32, tag="nsb")
                nc.tensor.matmul(psg[:, :P], blkq[p][:], blkq[p][:], start=True, stop=True)
                g = wk.tile([P, P], f32, tag="g")
                nc.scalar.activation(g[:], psg[:, :P], ACTF.Exp, scale=0.125)
                nc.vector.memset(g[0:64, 64:128], 0.0)
                nc.vector.memset(g[64:128, 0:64], 0.0)
                gs = wk.tile([P, 1], f32, tag="gs")
                nc.vector.reduce_sum(gs[:], g[:], axis=AX.X)
                gri = wk.tile([P, 1], f32, tag="gri")
                nc.vector.reciprocal(gri[:], gs[:])
                gd = nsp.tile([P, P], f32, tag=f"gd{p}", name=f"gd{p}")
                nc.vector.tensor_scalar_mul(gd[:], g[:], gri[:])

                pskt = nsps.tile([P, 256], f32, tag="nsb")
                nc.tensor.matmul(pskt[:, :P], gd[:], ident32[:], is_transpose=True)
                ktpad = nsp.tile([P, 256], f32r, tag=f"kt{p}")
                nc.vector.tensor_copy(ktpad[:, P:], zsrc[:])
                csum = wk.tile([P, 1], f32, tag="csum")
                nc.vector.tensor_scalar(
                    ktpad[:, :P], pskt[:, :P], 1.0, None, ALU.mult, ALU.add, accum_out=csum[:]
                )
                # partition_all_reduce only works at base partition 0:
                # separate the two heads into columns, pad with -1e30
                csep = wk.tile([P, 2], f32, tag="csep")
                nc.vector.memset(csep[:], -1e30)
                nc.vector.tensor_copy(csep[0:64, 0:1], csum[0:64, :])
                nc.vector.tensor_copy(csep[64:128, 1:2], csum[64:128, :])
                cmax = wk.tile([P, 2], f32, tag="cmax")
                nc.gpsimd.partition_all_reduce(
                    cmax[:], csep[:], P, bass_isa.ReduceOp.max
                )
                if global_scale:
                    nc.vector.tensor_copy(gstage[0:1, 2 * p:2 * p + 2], cmax[0:1, 0:2])
                    sv = None
                else:
                    sv = wk.tile([P, 1], f32, tag="sv")
                    nc.vector.reciprocal(sv[0:64, :], cmax[0:64, 0:1])
                    nc.vector.reciprocal(sv[64:128, :], cmax[64:128, 1:2])
                vstate.append([ktpad, sv, gd])

            if global_scale:
                gmax = pers.tile([1, 1], f32, tag="gmax")
                nc.vector.reduce_max(gmax[:], gstage[:], axis=AX.X)
                nc.sync.dma_start(cc_in.ap(), gmax[:])
                nc.gpsimd.collective_compute(
                    "AllReduce", ALU.max, [list(range(8))],
                    ins=[cc_in.ap().opt()], outs=[cc_out.ap().opt()],
                )
                gback = pers.tile([1, 1], f32, tag="gback")
                nc.sync.dma_start(gback[:], cc_out.ap())
                psb = nsps.tile([P, 256], f32, tag="nsb")
                nc.tensor.matmul(psb[:, 0:1], ones_row[:], gback[:], start=True, stop=True)
                sv_g = pers.tile([P, 1], f32, tag="sv_g")
                nc.vector.reciprocal(sv_g[:], psb[:, 0:1])

            for p in range(NPAIR):
                ktpad, sv, gd = vstate[p]
                if global_scale:
                    sv = sv_g
                v0 = nsp.tile([P, 256], f32r, tag=f"v{p}", name=f"v0_{p}")
                nc.vector.tensor_copy(v0[:, P:], zsrc[:])
                nc.vector.tensor_scalar_mul(v0[:, :P], ktpad[:, :P], sv[:])
                # V0^T = s*K directly (s constant within each head block)
                vt0 = nsp.tile([P, 256], f32r, tag=f"vt{p}", name=f"vt0_{p}")
                nc.vector.tensor_copy(vt0[:, P:], zsrc[:])
                nc.vector.tensor_scalar_mul(vt0[:, :P], gd[:], sv[:])
                vstate[p] = [ktpad, v0, vt0]

            # Newton-Schulz iterations (fp32r, right halves stay zero).
            # it-outer so the four independent pair-chains pipeline.
            vcur = [list(vstate[p]) for p in range(NPAIR)]
            for it in range(NS_ITERS):
                for p in range(NPAIR):
                    pool_a, tag_a = nsps, "nsb"
                    pool_b, tag_b = nsps, "nsb"
                    ktpad, v, vt = vcur[p]
                    pskv = pool_a.tile([P, 256], f32, tag=tag_a, name=f"pskv{p}_{it}")
                    nc.tensor.matmul(pskv[:], ktpad[:, :P], v[:], start=True, stop=True)
                    pskvt = pool_b.tile([P, 256], f32, tag=tag_b, name=f"pskvt{p}_{it}")
                    nc.tensor.matmul(pskvt[:], v[:, :P], ktpad[:], start=True, stop=True)
                    kvt = nsp.tile([P, 256], f32r, tag=f"kvt{p}", name=f"kvt{p}_{it}")
                    nc.vector.tensor_copy(kvt[:], pskvt[:])
                    a1 = nsp.tile([P, 256], f32r, tag=f"a1{p}", name=f"a1{p}_{it}")
                    nc.vector.tensor_tensor(a1[:], i7[:], pskv[:], ALU.subtract)
                    psa2 = pool_a.tile([P, 256], f32, tag=tag_a, name=f"psa2{p}_{it}")
                    nc.tensor.matmul(psa2[:], kvt[:, :P], a1[:], start=True, stop=True)
                    a3 = nsp.tile([P, 256], f32r, tag=f"a3{p}", name=f"a3{p}_{it}")
                    nc.vector.tensor_tensor(a3[:], i15[:], psa2[:], ALU.subtract)
                    psa4 = pool_b.tile([P, 256], f32, tag=tag_b, name=f"psa4{p}_{it}")
                    nc.tensor.matmul(psa4[:], kvt[:, :P], a3[:], start=True, stop=True)
                    a5 = nsp.tile([P, 256], f32r, tag=f"a5{p}", name=f"a5{p}_{it}")
                    nc.vector.tensor_tensor(a5[:], i13[:], psa4[:], ALU.subtract)
                    if it < NS_ITERS - 1:
                        psv = pool_a.tile([P, 256], f32, tag=tag_a, name=f"psv{p}_{it}")
                        nc.tensor.matmul(psv[:], vt[:, :P], a5[:], start=True, stop=True)
                        vn = nsp.tile([P, 256], f32r, tag=f"v{p}", name=f"vn{p}_{it}")
                        nc.vector.tensor_scalar_mul(vn[:], psv[:], 0.25)
                    else:
                        # v unused after the last iteration (W needs only vt)
                        vn = vcur[p][1]
                    psvt2 = pool_b.tile([P, 256], f32, tag=tag_b, name=f"psvt2{p}_{it}")
                    nc.tensor.matmul(psvt2[:], a5[:, :P], vt[:], start=True, stop=True)
                    vtn = nsp.tile([P, 256], f32r, tag=f"vt{p}", name=f"vtn{p}_{it}")
                    nc.vector.tensor_scalar_mul(vtn[:], psvt2[:], 0.25)
                    vcur[p] = [ktpad, vn, vtn]
            for p in range(NPAIR):
                vstate[p] = list(vcur[p])

            # ST = exp(blkQ^T @ QKVT / 8); c partials via accum_out
            cparts = []
            for p in range(NPAIR):
                cp = pers.tile([P, NCH8], f32, tag=f"cpart{p}")
                cparts.append(cp)
                for c in range(NCH8):
                    psst = trps.tile([P, 512], f32, tag="trp")
                    nc.tensor.matmul(
                        psst[:], blkq[p][:], qkvt[:, p, c * 512:(c + 1) * 512],
                        start=True, stop=True,
                    )
                    nc.scalar.activation(
                        st[:, p, c * 512:(c + 1) * 512], psst[:], ACTF.Exp,
                        scale=0.125, accum_out=cp[:, c:c + 1],
                    )

            # token-chunk loop: transposes + S-normal + M accumulation
            rv = pers.tile([P, 2 * NPAIR, TCH], f32, tag="rv")
            mbank = [mps.tile([P, 512], f32, tag=f"mb{q}", name=f"mb{q}") for q in range(2)]
            for c in range(TCH):
                tsl = slice(c * P, (c + 1) * P)
                psq = trps.tile([P, 512], f32r, tag="trp")
                for p in range(NPAIR):
                    nc.tensor.matmul(
                        psq[:, p * P:(p + 1) * P], qkvt[:, p, tsl], identr[:],
                        is_transpose=True, start=(p == 0), stop=(p == NPAIR - 1),
                        skip_group_check=True,
                    )
                qnb = snp.tile([P, 512], f32r, tag="qnb", name=f"qnb_{c}")
                nc.scalar.copy(qnb[:], psq[:])
                qn = [qnb[:, 0:256], qnb[:, 256:512]]
                pss = trps.tile([P, 512], f32r, tag="trp")
                for p in range(NPAIR):
                    nc.tensor.matmul(
                        pss[:, p * P:(p + 1) * P], st[:, p, tsl], identr[:],
                        is_transpose=True, start=(p == 0), stop=(p == NPAIR - 1),
                        skip_group_check=True,
                    )
                sn = [snp.tile([P, P], f32r, tag=f"sn{p}", name=f"sn{p}_{c}") for p in range(NPAIR)]
                for p in range(NPAIR):
                    nc.vector.tensor_scalar(
                        sn[p][:, 0:64], pss[:, p * P:p * P + 64], 1.0, None,
                        ALU.mult, ALU.add, accum_out=rv[:, 2 * p, c:c + 1],
                    )
                    nc.vector.tensor_scalar(
                        sn[p][:, 64:128], pss[:, p * P + 64:(p + 1) * P], 1.0, None,
                        ALU.mult, ALU.add, accum_out=rv[:, 2 * p + 1, c:c + 1],
                    )
                for q in range(2):
                    for j in range(2):
                        p = 2 * q + j
                        nc.tensor.matmul(
                            mbank[q][:, j * 256:(j + 1) * 256], sn[p][:], qn[q],
                            start=(c == 0 and j == 0),
                            stop=(c == TCH - 1 and j == 1),
                            skip_group_check=True,
                        )

            nc.vector.reciprocal(rv[:], rv[:])

            # W = V6 @ (diag(1/c) M)
            wpads = []
            for p in range(NPAIR):
                q, j = divmod(p, 2)
                cs = wk.tile([P, 1], f32, tag="cs")
                nc.vector.reduce_sum(cs[:], cparts[p][:], axis=AX.X)
                cinv = wk.tile([P, 1], f32, tag="cinv")
                nc.vector.reciprocal(cinv[:], cs[:])
                dvp = wk.tile([P, 256], f32r, tag="dvp")
                nc.vector.tensor_copy(dvp[:, P:], zsrc[:])
                nc.vector.tensor_scalar_mul(
                    dvp[:, :P], mbank[q][:, j * 384:j * 384 + P], cinv[:]
                )
                # zero cross-head blocks (garbage from the paired-rhs M matmul)
                nc.vector.tensor_copy(dvp[0:64, 64:128], zsrc[0:64, 0:64])
                nc.vector.tensor_copy(dvp[64:128, 0:64], zsrc[0:64, 0:64])
                psw = nsps.tile([P, 256], f32, tag="nsb")
                _, v6, vt6 = vstate[p]
                nc.tensor.matmul(psw[:], vt6[:, :P], dvp[:], start=True, stop=True)
                wpad = pers.tile([P, 256], f32r, tag=f"wpad{p}")
                nc.vector.tensor_copy(wpad[:], psw[:])
                wpads.append(wpad)

            # final: out = diag(1/r) S W  (2 pairs packed per psum bank)
            for c in range(TCH):
                tsl = slice(c * P, (c + 1) * P)
                for q in range(2):
                    pso = trps.tile([P, 512], f32, tag="trp", name=f"pso{q}_{c}")
                    for j in range(2):
                        p = 2 * q + j
                        nc.tensor.matmul(
                            pso[:, j * 256:j * 256 + 256], st[:, p, tsl], wpads[p][:],
                            start=(j == 0), stop=(j == 1), skip_group_check=True,
                        )
                    ot = wk.tile([P, 256], f32, tag="ot", name=f"ot{q}_{c}")
                    nc.vector.tensor_tensor(
                        ot[:].rearrange("p (b h d) -> p b h d", h=2, d=64),
                        pso[:].rearrange("p (b n) -> p b n", n=256)[:, :, 0:128]
                            .rearrange("p b (h d) -> p b h d", d=64),
                        rv[:, 4 * q:4 * q + 4, c:c + 1]
                            .rearrange("p (b h) one -> p b h one", h=2)
                            .to_broadcast([P, 2, 2, 64]),
                        ALU.mult,
                    )
                    nc.sync.dma_start(out_d[tsl, q * 256:(q + 1) * 256], ot[:])

    nc.compile()
    return nc


def _get_nc():
    if "nc" not in _CACHE:
        _CACHE["nc"] = _build()
    return _CACHE["nc"]


def kernel(X, Wq, bq):
    from concourse.bass_utils import run_bass_kernel_spmd

    nc = _get_nc()
    B, E, n = X.shape
    H = Wq.shape[0]
    in_maps = []
    for core in range(8):
        b = core // 2
        h0 = 8 * (core % 2)
        wq_c = Wq[h0:h0 + 8]                      # [8, 64, 1024]
        wqt_c = np.ascontiguousarray(wq_c.transpose(2, 0, 1).reshape(E, 512))
        bias_c = np.ascontiguousarray(bq[h0:h0 + 8].reshape(512))
        in_maps.append({
            "X": np.ascontiguousarray(X[b]),
            "WqT": wqt_c,
            "bias": bias_c,
        })
    res = run_bass_kernel_spmd(nc, in_maps, core_ids=list(range(8)))
    out = np.empty((B, H, n, 64), dtype=np.float32)
    for core in range(8):
        b = core // 2
        h0 = 8 * (core % 2)
        oc = res.results[core]["out"].reshape(n, 8, 64)
        out[b, h0:h0 + 8] = oc.transpose(1, 0, 2)
    return out



# revision 25
# speedup vs baseline: 2.0313x; 2.0313x over previous
"""CoNystromAttention Trainium2 kernel.

Shard: 8 cores = 4 batches x 2 head-groups (8 heads each). Per core:
one batch b, 8 heads organized as 4 "pairs" (2 heads = 128 partitions).

Math (reference, with Q=K=V=QKV):
  QKV = X[b].T @ Wq[h].T + bq[h]                       [n=4096, d=64]
  Qt  = window-mean(QKV, 64)                           [m=64, d]
  S   = exp(QKV @ Qt.T / 8)     (Beta; Delta = S.T)    [n, m]
  G   = exp(Qt @ Qt.T / 8)
  GD  = G / rowsum(G);  V6 = newton_schulz(GD, 6)      (pinv)
  out = diag(1/r) S V6 diag(1/c) S.T QKV,  r=rowsum(S), c=colsum(S)

Projection matmuls in f32r (DMA'd straight into f32r tiles); everything
downstream (S, transposes, M, NS, final) in bf16.  The NS operand K is
error-compensated as gd_hi + gd_lo (two accumulating bf16 matmuls) so
the iteration inverts GD at ~f32 precision.  NS init scale uses the
per-core max (8 heads) instead of the reference's global max; since GD
is row-normalized the rowsum term is exactly 1, so scale=1/max colsum.
"""

import numpy as np

P = 128
N_TOK = 4096
EMBED = 1024
NPAIR = 4            # head-pairs per core (8 heads)
ECH = EMBED // P     # 8 contraction chunks
XCH = 512            # projection chunk (tokens)
NCHP = N_TOK // XCH  # 8 projection chunks
NCH8 = N_TOK // 512  # 8 ST chunks of 512
TCH = N_TOK // P     # 32 token chunks of 128
NS_ITERS = 6

_CACHE = {}


def _build(**_ignored):
    import concourse.mybir as mybir
    from concourse import bacc, bass_isa
    from concourse.tile import TileContext
    from concourse.masks import make_identity

    f32 = mybir.dt.float32
    f32r = mybir.dt.float32r
    bf16 = mybir.dt.bfloat16
    ALU = mybir.AluOpType
    ACTF = mybir.ActivationFunctionType
    AX = mybir.AxisListType

    nc = bacc.Bacc("TRN2", target_bir_lowering=False, debug=False)
    X = nc.dram_tensor("X", [EMBED, N_TOK], f32, kind="ExternalInput")
    WqT = nc.dram_tensor("WqT", [EMBED, 512], f32, kind="ExternalInput")
    bias = nc.dram_tensor("bias", [512], f32, kind="ExternalInput")
    out_d = nc.dram_tensor("out", [N_TOK, 512], f32, kind="ExternalOutput")

    with TileContext(nc) as tc, (
        tc.tile_pool(name="big", bufs=1)
    ) as big, tc.tile_pool(name="pers", bufs=1) as pers, tc.tile_pool(
        name="nsv", bufs=2
    ) as nsp:
        # ---------------- persistent tiles ----------------
        ident32 = pers.tile([P, P], f32, tag="ident32")
        make_identity(nc, ident32[:])
        identb = pers.tile([P, P], bf16, tag="identb")
        nc.vector.tensor_copy(identb[:], ident32[:])
        iw = []
        for v in (7.0, 15.0, 13.0):
            t = pers.tile([P, NPAIR, P], f32, tag=f"i{int(v)}")
            nc.vector.tensor_scalar_mul(
                t[:],
                ident32[:].rearrange("p (q c) -> p q c", q=1).to_broadcast(
                    [P, NPAIR, P]
                ),
                v,
            )
            iw.append(t)
        i7w, i15w, i13w = iw
        bias_t = pers.tile([P, NPAIR], f32, tag="bias")
        nc.sync.dma_start(bias_t[:], bias.rearrange("(f p) -> p f", p=P))
        ones2b = pers.tile([P, 2], bf16, tag="ones2b")
        nc.vector.memset(ones2b[:], 0.0)
        nc.vector.memset(ones2b[0:64, 0:1], 1.0)
        nc.vector.memset(ones2b[64:128, 1:2], 1.0)
        onescol = pers.tile([P, 1], bf16, tag="onescol")
        nc.vector.memset(onescol[:], 1.0)
        qsum4 = pers.tile([P, NPAIR, 64], f32, tag="qsum4")
        qkvt = big.tile([P, NPAIR, N_TOK], bf16, tag="qkvt")
        st = big.tile([P, NPAIR, N_TOK], bf16, tag="st")

        # ---------------- phase 1: projection ----------------
        with (
            tc.tile_pool(name="wq", bufs=1) as wqp,
            tc.tile_pool(name="x", bufs=8) as xpool,
            tc.tile_pool(name="pp", bufs=8, space="PSUM") as pp,
        ):
            wqre = WqT.rearrange("(eo p) hd -> p eo hd", p=P).bitcast(f32r)
            xre = X.rearrange("(eo p) n -> p eo n", p=P).bitcast(f32r)

            xtiles = {}

            def load_x(c, half):
                csl = slice(c * XCH, (c + 1) * XCH)
                xt = xpool.tile([P, 2, XCH], f32r, tag="xt")
                nc.sync.dma_start(
                    xt[:], xre[:, half * 2:(half + 1) * 2, csl]
                )
                xtiles.setdefault(c, []).append(xt)

            # startup order: interleave the first X chunk's eo-pairs with
            # the matching wq chunks so matmul e can start as soon as its
            # own two DMAs land (DMA queue drains in emission order).
            wqe = []

            def load_wq(e):
                t = wqp.tile([P, 512], f32r, tag=f"wq{e}")
                nc.sync.dma_start(t[:], wqre[:, e, :])
                wqe.append(t)

            for half in range(4):
                load_x(0, half)
                load_wq(2 * half)
                load_wq(2 * half + 1)
            for half in range(4):
                load_x(1, half)

            for c in range(NCHP):
                csl = slice(c * XCH, (c + 1) * XCH)
                xrs = xtiles.pop(c)
                for p in range(NPAIR):
                    ps = pp.tile([P, XCH], f32, tag="proj")
                    for e in range(ECH):
                        nc.tensor.matmul(
                            ps[:],
                            wqe[e][:, p * P:(p + 1) * P],
                            xrs[e // 2][:, e % 2, :],
                            start=(e == 0),
                            stop=(e == ECH - 1),
                        )
                    # PSUM -> SBUF bf16 with bias fused; alternate DVE/Act
                    if p % 2 == 0:
                        nc.vector.tensor_scalar_add(
                            qkvt[:, p, csl], ps[:], bias_t[:, p:p + 1]
                        )
                    else:
                        nc.scalar.activation(
                            qkvt[:, p, csl], ps[:], ACTF.Identity,
                            bias=bias_t[:, p:p + 1],
                        )
                if c + 2 < NCHP:
                    for half in range(4):
                        load_x(c + 2, half)
                # landmark partial sums (post-bias): 8 windows/chunk/pair
                nw = XCH // 64
                nc.vector.tensor_reduce(
                    qsum4[:, :, c * nw:(c + 1) * nw],
                    qkvt[:, :, csl].rearrange("p q (w t) -> p q w t", t=64),
                    axis=AX.X, op=ALU.add,
                )

        # ---------------- phase 2+: everything else ----------------
        with (
            tc.tile_pool(name="wk", bufs=4) as wk,
            tc.tile_pool(name="nsps", bufs=2, space="PSUM") as nsps,
            tc.tile_pool(name="stps", bufs=2, space="PSUM") as stps,
            tc.tile_pool(name="trp", bufs=2, space="PSUM") as trp,
            tc.tile_pool(name="mps", bufs=1, space="PSUM") as mps,
            tc.tile_pool(name="cps", bufs=1, space="PSUM") as cps,
        ):
            # landmarks: blkq = qsum/64 (bias already included), block-diag
            blkq = pers.tile([P, NPAIR, P], bf16, tag="blkq")
            nc.vector.memset(blkq[:], 0.0)
            nc.vector.tensor_scalar_mul(
                blkq[0:64, :, 0:64], qsum4[0:64, :, :], 1.0 / 64
            )
            nc.vector.tensor_scalar_mul(
                blkq[64:128, :, 64:128], qsum4[64:128, :, :], 1.0 / 64
            )

            # Gamma -> GD (row-normalized), all 4 pairs in one bank
            psg = nsps.tile([P, NPAIR, P], f32, tag="nsb", name="psg")
            for p in range(NPAIR):
                nc.tensor.matmul(
                    psg[:, p, :], blkq[:, p, :], blkq[:, p, :],
                    start=(p == 0), stop=(p == NPAIR - 1),
                    skip_group_check=True,
                )
            g_all = wk.tile([P, NPAIR, P], f32, tag="g")
            nc.scalar.activation(g_all[:], psg[:], ACTF.Exp, scale=0.125)
            gs4 = wk.tile([P, NPAIR], f32, tag="gs4")
            nc.vector.tensor_reduce(
                gs4[0:64, :], g_all[0:64, :, 0:64], axis=AX.X, op=ALU.add
            )
            nc.vector.tensor_reduce(
                gs4[64:128, :], g_all[64:128, :, 64:128], axis=AX.X, op=ALU.add
            )
            gri = wk.tile([P, NPAIR], f32, tag="gri")
            nc.vector.reciprocal(gri[:], gs4[:])
            gdf = wk.tile([P, NPAIR, P], f32, tag="gdf")
            nc.vector.memset(gdf[:], 0.0)
            nc.vector.tensor_tensor(
                gdf[0:64, :, 0:64], g_all[0:64, :, 0:64],
                gri[0:64, :].rearrange("p (q o) -> p q o", o=1).to_broadcast(
                    [64, NPAIR, 64]
                ),
                ALU.mult,
            )
            nc.vector.tensor_tensor(
                gdf[64:128, :, 64:128], g_all[64:128, :, 64:128],
                gri[64:128, :].rearrange("p (q o) -> p q o", o=1).to_broadcast(
                    [64, NPAIR, 64]
                ),
                ALU.mult,
            )
            # compensated K: gd (hi) + gd_lo so NS inverts GD at ~f32 precision
            gd = pers.tile([P, NPAIR, P], bf16, tag="gd")
            nc.vector.tensor_copy(gd[:], gdf[:])
            gd_lo = pers.tile([P, NPAIR, P], bf16, tag="gd_lo")
            nc.vector.scalar_tensor_tensor(
                gd_lo[:], gd[:], -1.0, gdf[:], ALU.mult, ALU.add
            )

            # ---- fused: ST exp + token loop + NS iterations ----
            mbank = mps.tile([P, NPAIR, P], f32, tag="mbank")
            # one bank holds all 32 chunks' per-token S rowsums (rall);
            # the Delta rowsums come from the exps' accum_out (cparts)
            combo = cps.tile([P, TCH * NPAIR * 2], f32, tag="rall")
            rall = combo[:].rearrange("p (c q h) -> p c q h", q=NPAIR, h=2)
            cparts = pers.tile([P, NPAIR, NCH8], f32, tag="cparts")

            def emit_st_pair(j, p):
                jsl = slice(j * 512, (j + 1) * 512)
                psst = stps.tile([P, 512], f32, tag="stb", name=f"psst{j}_{p}")
                nc.tensor.matmul(
                    psst[:], blkq[:, p, :], qkvt[:, p, jsl],
                    start=True, stop=True,
                )
                nc.scalar.activation(
                    st[:, p, jsl], psst[:], ACTF.Exp, scale=0.125,
                    accum_out=cparts[:, p, j:j + 1],
                )

            def emit_token_chunk(c):
                tsl = slice(c * P, (c + 1) * P)
                tr = trp.tile([P, 2, NPAIR, P], bf16, tag="tr", name=f"tr{c}")
                for i, src in enumerate((qkvt, st)):
                    for p in range(NPAIR):
                        nc.tensor.matmul(
                            tr[:, i, p, :], src[:, p, tsl], identb[:],
                            is_transpose=True,
                            start=(i == 0 and p == 0),
                            stop=(i == 1 and p == NPAIR - 1),
                            skip_group_check=True,
                        )
                trn = wk.tile([P, 2, NPAIR, P], bf16, tag="trn", name=f"trn{c}")
                nc.vector.tensor_copy(trn[:], tr[:])
                qnb = trn[:, 0]
                snb = trn[:, 1]
                for p in range(NPAIR):
                    nc.tensor.matmul(
                        mbank[:, p, :], snb[:, p, :], qnb[:, p, :],
                        start=(c == 0 and p == 0),
                        stop=(c == TCH - 1 and p == NPAIR - 1),
                        skip_group_check=True,
                    )
                    nc.tensor.matmul(
                        rall[:, c, p, :], st[:, p, tsl], ones2b[:],
                        start=(c == 0 and p == 0),
                        stop=(c == TCH - 1 and p == NPAIR - 1),
                        skip_group_check=True,
                    )

            def ns_mm(out, mk, it_name):
                """out[:, p, :] = sum over (hi, lo) accumulated matmuls."""
                ops = []
                for p in range(NPAIR):
                    ops.append(mk(p))
                n = 0
                for p, pairs in enumerate(ops):
                    for lhsT, rhs in pairs:
                        n += 1
                        nc.tensor.matmul(
                            out[:, p, :], lhsT, rhs,
                            start=(n == 1),
                            stop=(n == sum(len(o) for o in ops)),
                            skip_group_check=True,
                        )

            def emit_ns_part1(it, v_in, vt_in):
                pskv = nsps.tile([P, NPAIR, P], f32, tag="nsb", name=f"pskv{it}")
                ns_mm(pskv, lambda p: [
                    (ktr[:, p, :], v_in[:, p, :]),
                    (ktr_lo[:, p, :], v_in[:, p, :]),
                ], it)
                pskvt = nsps.tile([P, NPAIR, P], f32, tag="nsb", name=f"pskvt{it}")
                ns_mm(pskvt, lambda p: [
                    (v_in[:, p, :], ktr[:, p, :]),
                    (v_in[:, p, :], ktr_lo[:, p, :]),
                ], it)
                kvt = nsp.tile([P, NPAIR, P], bf16, tag="kvt", name=f"kvt{it}")
                nc.scalar.copy(kvt[:], pskvt[:])
                a1 = nsp.tile([P, NPAIR, P], bf16, tag="a1", name=f"a1_{it}")
                nc.vector.scalar_tensor_tensor(
                    a1[:], pskv[:], -1.0, i7w[:], ALU.mult, ALU.add
                )
                psa2 = nsps.tile([P, NPAIR, P], f32, tag="nsb", name=f"psa2{it}")
                ns_mm(psa2, lambda p: [(kvt[:, p, :], a1[:, p, :])], it)
                a3 = nsp.tile([P, NPAIR, P], bf16, tag="a3", name=f"a3_{it}")
                nc.vector.scalar_tensor_tensor(
                    a3[:], psa2[:], -1.0, i15w[:], ALU.mult, ALU.add
                )
                return kvt, a3

            def emit_ns_part2(it, v_in, vt_in, kvt, a3):
                psa4 = nsps.tile([P, NPAIR, P], f32, tag="nsb", name=f"psa4{it}")
                ns_mm(psa4, lambda p: [(kvt[:, p, :], a3[:, p, :])], it)
                a5 = nsp.tile([P, NPAIR, P], bf16, tag="a5", name=f"a5_{it}")
                nc.vector.scalar_tensor_tensor(
                    a5[:], psa4[:], -1.0, i13w[:], ALU.mult, ALU.add
                )
                if it < NS_ITERS - 1:
                    psv = nsps.tile([P, NPAIR, P], f32, tag="nsb", name=f"psv{it}")
                    ns_mm(psv, lambda p: [(vt_in[:, p, :], a5[:, p, :])], it)
                    vn = nsp.tile([P, NPAIR, P], bf16, tag="v", name=f"v{it + 1}")
                    nc.scalar.mul(vn[:], psv[:], 0.25)
                else:
                    vn = v_in
                psvt2 = nsps.tile([P, NPAIR, P], f32, tag="nsb", name=f"psvt2{it}")
                ns_mm(psvt2, lambda p: [(a5[:, p, :], vt_in[:, p, :])], it)
                vtn = nsp.tile([P, NPAIR, P], bf16, tag="vt", name=f"vt{it + 1}")
                nc.scalar.mul(vtn[:], psvt2[:], 0.25)
                return vn, vtn

            # early ST block 0 + first two token chunks overlap the
            # serial NS-scale / NS-init chain below
            for p in range(NPAIR):
                emit_st_pair(0, p)
            emit_token_chunk(0)
            emit_token_chunk(1)

            # NS scale (per-core): rowsums of GD are exactly 1, so
            # scale = 1 / max colsum over the core's 8 heads.
            psc = nsps.tile([P, NPAIR, P], f32, tag="nsb", name="psc")
            for p in range(NPAIR):
                nc.tensor.matmul(
                    psc[0:2, p, :], ones2b[:], gd[:, p, :],
                    start=(p == 0), stop=(p == NPAIR - 1),
                    skip_group_check=True,
                )
            cm2 = wk.tile([2, 1], f32, tag="cm2")
            nc.vector.reduce_max(
                cm2[:], psc[0:2].rearrange("p q c -> p (q c)"), axis=AX.X
            )
            csep = wk.tile([P, 1], f32, tag="csep")
            nc.vector.memset(csep[:], -1e30)
            nc.vector.tensor_copy(csep[0:2, :], cm2[:])
            cmax = wk.tile([P, 1], f32, tag="cmax")
            nc.gpsimd.partition_all_reduce(
                cmax[:], csep[:], P, bass_isa.ReduceOp.max
            )
            sv = pers.tile([P, 1], f32, tag="sv")
            nc.vector.reciprocal(sv[:], cmax[:])

            # NS init: V0 = s*GD^T, V0^T = s*GD, K^T = GD^T (+lo residual)
            pskt = trp.tile([P, 2, NPAIR, P], bf16, tag="tr", name="pskt")
            for i, src in enumerate((gd, gd_lo)):
                for p in range(NPAIR):
                    nc.tensor.matmul(
                        pskt[:, i, p, :], src[:, p, :], identb[:],
                        is_transpose=True,
                        start=(i == 0 and p == 0),
                        stop=(i == 1 and p == NPAIR - 1),
                        skip_group_check=True,
                    )
            ktr2 = pers.tile([P, 2, NPAIR, P], bf16, tag="ktr2")
            nc.vector.tensor_copy(ktr2[:], pskt[:])
            ktr = ktr2[:, 0]
            ktr_lo = ktr2[:, 1]
            v_cur = nsp.tile([P, NPAIR, P], bf16, tag="v", name="v0")
            nc.vector.tensor_scalar_mul(v_cur[:], pskt[:, 0], sv[:])
            vt_cur = nsp.tile([P, NPAIR, P], bf16, tag="vt", name="vt0")
            nc.vector.tensor_scalar_mul(vt_cur[:], gd[:], sv[:])

            ns_state = None
            for j in range(NCH8):
                for i, c in enumerate(range(4 * j, 4 * j + 4)):
                    if c >= 2:
                        emit_token_chunk(c)
                    if j + 1 < NCH8:
                        emit_st_pair(j + 1, i)
                    if i == 1 and j < NS_ITERS:
                        ns_state = emit_ns_part1(j, v_cur, vt_cur)
                    elif i == 3 and j < NS_ITERS:
                        v_cur, vt_cur = emit_ns_part2(
                            j, v_cur, vt_cur, *ns_state
                        )

            # ---- precompute 1/r for every token (one wide reciprocal) ----
            rinv_all = pers.tile([P, TCH, NPAIR, 2], f32, tag="rinv_all")
            nc.vector.reciprocal(
                rinv_all[:].rearrange("p c q h -> p (c q h)"), combo[:]
            )

            # ---- W = V6 @ diag(1/c) M ----
            csum = wk.tile([P, NPAIR], f32, tag="csum")
            nc.vector.tensor_reduce(
                csum[:], cparts[:], axis=AX.X, op=ALU.add
            )
            cinv = wk.tile([P, NPAIR], f32, tag="cinv")
            nc.vector.reciprocal(cinv[:], csum[:])
            dvp = wk.tile([P, NPAIR, P], bf16, tag="dvp")
            nc.vector.memset(dvp[:], 0.0)
            nc.vector.tensor_tensor(
                dvp[0:64, :, 0:64], mbank[0:64, :, 0:64],
                cinv[0:64, :].rearrange("p (q o) -> p q o", o=1).to_broadcast(
                    [64, NPAIR, 64]
                ),
                ALU.mult,
            )
            nc.vector.tensor_tensor(
                dvp[64:128, :, 64:128], mbank[64:128, :, 64:128],
                cinv[64:128, :].rearrange("p (q o) -> p q o", o=1).to_broadcast(
                    [64, NPAIR, 64]
                ),
                ALU.mult,
            )
            psw = nsps.tile([P, NPAIR, P], f32, tag="nsb", name="psw")
            for p in range(NPAIR):
                nc.tensor.matmul(
                    psw[:, p, :], vt_cur[:, p, :], dvp[:, p, :],
                    start=(p == 0), stop=(p == NPAIR - 1),
                    skip_group_check=True,
                )
            wpad = pers.tile([P, NPAIR, P], bf16, tag="wpad")
            nc.vector.tensor_copy(wpad[:], psw[:])

        # ---------------- final: out = diag(1/r) S W ----------------
        with (
            tc.tile_pool(name="fin", bufs=8) as fin,
            tc.tile_pool(name="pso", bufs=6, space="PSUM") as psop,
        ):
            for c in range(TCH):
                tsl = slice(c * P, (c + 1) * P)
                pso = psop.tile([P, NPAIR, P], f32, tag="pso", name=f"pso{c}")
                for p in range(NPAIR):
                    nc.tensor.matmul(
                        pso[:, p, :], st[:, p, tsl], wpad[:, p, :],
                        start=(p == 0), stop=(p == NPAIR - 1),
                        skip_group_check=True,
                    )
                ot = fin.tile([P, NPAIR, P], f32, tag="ot", name=f"ot{c}")
                nc.vector.tensor_tensor(
                    ot[:].rearrange("p q (h d) -> p q h d", d=64),
                    pso[:].rearrange("p q (h d) -> p q h d", d=64),
                    rinv_all[:, c].rearrange(
                        "p q (h o) -> p q h o", o=1
                    ).to_broadcast([P, NPAIR, 2, 64]),
                    ALU.mult,
                )
                nc.sync.dma_start(
                    out_d[tsl, :], ot[:].rearrange("p q c -> p (q c)")
                )

    nc.compile()
    return nc


def _get_nc():
    if "nc" not in _CACHE:
        _CACHE["nc"] = _build()
    return _CACHE["nc"]


def kernel(X, Wq, bq):
    from concourse.bass_utils import run_bass_kernel_spmd

    nc = _get_nc()
    B, E, n = X.shape
    H = Wq.shape[0]
    in_maps = []
    for core in range(8):
        b = core // 2
        h0 = 8 * (core % 2)
        wq_c = Wq[h0:h0 + 8]                      # [8, 64, 1024]
        wqt_c = np.ascontiguousarray(wq_c.transpose(2, 0, 1).reshape(E, 512))
        bias_c = np.ascontiguousarray(bq[h0:h0 + 8].reshape(512))
        in_maps.append({
            "X": np.ascontiguousarray(X[b]),
            "WqT": wqt_c,
            "bias": bias_c,
        })
    res = run_bass_kernel_spmd(nc, in_maps, core_ids=list(range(8)))
    out = np.empty((B, H, n, 64), dtype=np.float32)
    for core in range(8):
        b = core // 2
        h0 = 8 * (core % 2)
        oc = res.results[core]["out"].reshape(n, 8, 64)
        out[b, h0:h0 + 8] = oc.transpose(1, 0, 2)
    return out
